# revision 26
# baseline (speedup 1.0000x reference)
"""Trainium2 Bass kernel for BERelativeSelfMultiheadAttn.

Strategy (data-parallel over batch B=8, one batch per NeuronCore):
  - Host folds the BatchEnsemble scale vectors r_*/s_* and the attention
    scale 1/sqrt(hd) into per-batch effective weight matrices (bf16),
    transposes x/pos, and packs weights so every DMA row is a 2KB
    contiguous chunk (fat descriptors).
  - On device, Q/K/rk are computed transposed ([feature, t]) so the
    score matmuls need no on-chip transposes; V is computed in [t, feature]
    layout with an extra ones column that yields the softmax normalizer Z
    for free from the PV matmul.
  - The relative shift is done exactly via the classic flat-buffer trick:
    bd [T, Lr] is written to DRAM as [T, 1+Lr] rows (zero in col 0); the
    shifted matrix is flat[T : T+T*T].reshape(T, T).  It is read back with a
    transposing DMA (bf16) and accumulated into the score PSUM with an
    identity matmul.
  - Softmax skips the max-subtraction (scores are O(1); exp cannot
    overflow).  The normalizer 1/Z is computed with a DVE reciprocal and
    broadcast across partitions by GpSimd (no activation-table swaps, no
    tensor-engine involvement); the context multiply is deferred by one
    head pair so no engine ever stalls on it.
  - Context is stored packed in head pairs [128, T] so the output
    projection runs K=128 matmuls.
  - DMA descriptor generation is spread across the two HWDGE queues:
    transposed reads on Sync, bd writes split Sync/Scalar, weight/x loads
    on Scalar.  Shifted-row reads for pair p are issued before pair p+1's
    bd writes to avoid head-of-line blocking.
"""

import numpy as np
import ml_dtypes

import concourse.bass as bass
import concourse.mybir as mybir
import concourse.tile as tile
from concourse import bacc
from concourse.bass_utils import run_bass_kernel_spmd

F32 = mybir.dt.float32
BF16 = mybir.dt.bfloat16
Act = mybir.ActivationFunctionType
Alu = mybir.AluOpType

P = 128


def build_program(T=1024, H=1024, heads=16, num_devices=8, enable_asserts=False):
    hd = H // heads
    assert hd == 64, "layout assumes head dim 64"
    nT = T // P            # t/q/k/r 128-blocks
    nH = H // P            # hidden-feature 128-blocks
    hpb = P // hd          # heads per 128-block (2)
    npair = heads // hpb
    CHT = min(512, T)      # matmul N chunk along T
    nCT = T // CHT
    CHH = min(512, H)      # matmul N chunk along H (V features)
    nCH_ = H // CHH
    Lr = T

    nc = bacc.Bacc(
        "TRN2",
        target_bir_lowering=False,
        debug=False,
        enable_asserts=enable_asserts,
        num_devices=num_devices,
    )

    xT_d = nc.dram_tensor("xT", [H, T], BF16, kind="ExternalInput").ap()
    posT_d = nc.dram_tensor("posT", [H, Lr], BF16, kind="ExternalInput").ap()
    wq_d = nc.dram_tensor("wq", [nH, P, H], BF16, kind="ExternalInput").ap()
    wk_d = nc.dram_tensor("wk", [nH, P, H], BF16, kind="ExternalInput").ap()
    wp_d = nc.dram_tensor("wp", [nH, P, H], BF16, kind="ExternalInput").ap()
    wv_d = nc.dram_tensor("wv", [nH, P, H], BF16, kind="ExternalInput").ap()
    wo_d = nc.dram_tensor("wo", [nH, P, H], BF16, kind="ExternalInput").ap()
    bqrw_d = nc.dram_tensor("bqrw", [nH, P, 1], F32, kind="ExternalInput").ap()
    bk_d = nc.dram_tensor("bk", [nH, P, 1], F32, kind="ExternalInput").ap()
    bp_d = nc.dram_tensor("bp", [nH, P, 1], F32, kind="ExternalInput").ap()
    drr_d = nc.dram_tensor("drr", [nH, P, 1], F32, kind="ExternalInput").ap()
    bo_d = nc.dram_tensor("bo", [nH, P, 1], F32, kind="ExternalInput").ap()
    ident_d = nc.dram_tensor("ident", [P, P], BF16, kind="ExternalInput").ap()
    outT_d = nc.dram_tensor("outT", [H, T], F32, kind="ExternalOutput").ap()

    with tile.TileContext(nc) as tc:
        with (
            tc.tile_pool(name="const", bufs=1) as constp,
            tc.tile_pool(name="persist", bufs=1) as pp,
        ):
            # constants
            id_sb = constp.tile([P, P], BF16, tag="ident")
            nc.sync.dma_start(id_sb[:], ident_d[:])
            bqrw_t, bk_t, bp_t, drr_t, bo_t = [], [], [], [], []
            for jo in range(nH):
                for lst, d, nm in (
                    (bqrw_t, bqrw_d, "bqrw"),
                    (bk_t, bk_d, "bk"),
                    (bp_t, bp_d, "bp"),
                    (drr_t, drr_d, "drr"),
                    (bo_t, bo_d, "bo"),
                ):
                    t = constp.tile([P, 1], F32, tag=f"{nm}{jo}", name=f"{nm}_{jo}")
                    nc.sync.dma_start(t[:], d[jo])
                    lst.append(t)

            # persistent activation tensors
            rkT = [pp.tile([P, Lr], BF16, tag=f"rk{i}", name=f"rkT{i}")
                   for i in range(nH)]
            Qrw = [pp.tile([P, T], BF16, tag=f"qrw{i}", name=f"Qrw{i}")
                   for i in range(nH)]
            Kt = [pp.tile([P, T], BF16, tag=f"kt{i}", name=f"Kt{i}")
                  for i in range(nH)]
            Vsb = [pp.tile([P, heads * (hd + 1)], BF16, tag=f"v{i}", name=f"Vsb{i}")
                   for i in range(nT)]
            # context packed by head PAIR: pair pr rows 0:64 = head 2pr,
            # rows 64:128 = head 2pr+1  ->  K=128 output projection
            ctxp = [pp.tile([P, T], BF16, tag=f"ctx{pr}", name=f"ctxp{pr}")
                    for pr in range(npair)]

            # -------- Phases 1+2: rk, Q, K (transposed), V (direct) --------
            with (
                tc.tile_pool(name="w12", bufs=1) as wpool,
                tc.tile_pool(name="xp", bufs=1) as xpool,
                tc.tile_pool(name="ps12", bufs=3, space=bass.MemorySpace.PSUM) as psp2,
            ):
                # Preload everything up front (scalar HWDGE queue) so the
                # tensor engine never starves and its clock stays ramped.
                posT_sb = [xpool.tile([P, Lr], BF16, tag=f"pos{i}",
                                      name=f"posT{i}") for i in range(nH)]
                xT_sb = [xpool.tile([P, T], BF16, tag=f"x{i}", name=f"xT{i}")
                         for i in range(nH)]
                wp_sb = [wpool.tile([P, H], BF16, tag=f"wp{i}", name=f"wp{i}")
                         for i in range(nH)]
                wq_sb = [wpool.tile([P, H], BF16, tag=f"wq{i}", name=f"wq{i}")
                         for i in range(nH)]
                wk_sb = [wpool.tile([P, H], BF16, tag=f"wk{i}", name=f"wk{i}")
                         for i in range(nH)]
                wv_sb = [wpool.tile([P, H], BF16, tag=f"wv{i}", name=f"wv{i}")
                         for i in range(nH)]
                for kb in range(nH):
                    nc.sync.dma_start(posT_sb[kb][:], posT_d[kb * P:(kb + 1) * P, :])
                    nc.scalar.dma_start(wp_sb[kb][:], wp_d[kb])
                for kb in range(nH):
                    nc.sync.dma_start(xT_sb[kb][:], xT_d[kb * P:(kb + 1) * P, :])
                    nc.scalar.dma_start(wq_sb[kb][:], wq_d[kb])
                for kb in range(nH):
                    nc.scalar.dma_start(wk_sb[kb][:], wk_d[kb])
                    nc.scalar.dma_start(wv_sb[kb][:], wv_d[kb])

                # rk projection
                for jo in range(nH):
                    pss = psp2.tile([P, T], F32, tag="ps12", name="ps1t")
                    for kb in range(nH):
                        for c in range(nCT):
                            nc.tensor.matmul(
                                pss[:, c * CHT:(c + 1) * CHT],
                                wp_sb[jo][:, kb * P:(kb + 1) * P],
                                posT_sb[kb][:, c * CHT:(c + 1) * CHT],
                                start=(kb == 0),
                                stop=(kb == nH - 1),
                            )
                    with nc.allow_low_precision(reason="bf16 activations"):
                        nc.vector.tensor_scalar_add(
                            rkT[jo][:], pss[:], bp_t[jo][:])

                # Q / K projections
                for jo in range(nH):
                    for w_sb, bias_t, dst in (
                        (wq_sb, bqrw_t, Qrw),
                        (wk_sb, bk_t, Kt),
                    ):
                        pss = psp2.tile([P, T], F32, tag="ps12", name="ps2t")
                        for kb in range(nH):
                            for c in range(nCT):
                                nc.tensor.matmul(
                                    pss[:, c * CHT:(c + 1) * CHT],
                                    w_sb[jo][:, kb * P:(kb + 1) * P],
                                    xT_sb[kb][:, c * CHT:(c + 1) * CHT],
                                    start=(kb == 0),
                                    stop=(kb == nH - 1),
                                )
                        with nc.allow_low_precision(reason="bf16 activations"):
                            nc.vector.tensor_scalar_add(
                                dst[jo][:], pss[:], bias_t[jo][:])

                # V (direct layout [t, feature] with interleaved ones col)
                for ti in range(nT):
                    nc.vector.memset(Vsb[ti][:], 1.0)
                    psv = psp2.tile([P, H], F32, tag="ps12", name="psvt")
                    for kb in range(nH):
                        for c in range(nCH_):
                            nc.tensor.matmul(
                                psv[:, c * CHH:(c + 1) * CHH],
                                xT_sb[kb][:, ti * P:(ti + 1) * P],
                                wv_sb[kb][:, c * CHH:(c + 1) * CHH],
                                start=(kb == 0),
                                stop=(kb == nH - 1),
                            )
                    # strided copy PSUM -> interleaved [V_h | 1] layout
                    nc.vector.tensor_copy(
                        Vsb[ti][:].rearrange(
                            "p (h e) -> p h e", e=hd + 1)[:, :, 0:hd],
                        psv[:].rearrange("p (h d) -> p h d", d=hd),
                    )

            # Preload the output-projection weights (packed per jo, pair
            # tiles side by side) so phase 4 never waits on DMA.
            wo_sb = [pp.tile([P, H], BF16, tag=f"wo{jo}", name=f"wo{jo}")
                     for jo in range(nH)]
            for jo in range(nH):
                nc.scalar.dma_start(wo_sb[jo][:], wo_d[jo])

            # ---------------- Phase 3: per head-pair attention -------------
            # Software-pipelined: shifted-row reads of pair p are issued
            # first, then bd scores of pair p+1, then the attention compute
            # of pair p, so the PE never drains at the bd -> DRAM ->
            # shifted-read dependency and the sync queue never head-of-line
            # blocks on not-yet-ready bd writes.
            with (
                tc.tile_pool(name="qrr", bufs=2) as qrrp,
                tc.tile_pool(name="bdout", bufs=6) as bdoutp,
                tc.tile_pool(name="bdT", bufs=16) as bdTp,
                tc.tile_pool(name="pT", bufs=4) as pTp,
                tc.tile_pool(name="ctxs", bufs=5) as ctxsp,
                tc.tile_pool(name="zsm", bufs=2) as zsmp,
                tc.tile_pool(name="zbp", bufs=4) as zbp,
                tc.tile_pool(name="psS", bufs=3, space=bass.MemorySpace.PSUM) as psS,
                tc.tile_pool(name="psC", bufs=1, space=bass.MemorySpace.PSUM) as psC,
                tc.tile_pool(name="bdd", bufs=6, space=bass.MemorySpace.DRAM) as dramp,
            ):
                def shifted_view(bdd_h):
                    flat = bdd_h[:].rearrange("a b -> (a b)")
                    return flat[T:T + T * T].rearrange("(a b) -> a b", b=T)

                bdd_map = {}    # pr -> [dram tile per head]
                views_map = {}  # pr -> [shifted view per head]
                bdTs_map = {}   # pr -> [[bdT tiles kb=0..7] per head]

                def bd_step(pr, qrr_t, qi):
                    """bd raw scores for both heads of (pr, qi):
                    4 matmuls + 2 PSUM->bf16 casts + 2 flat-buffer writes."""
                    for h in range(hpb):
                        base = h * hd
                        psb = psS.tile([P, T], F32, tag="s", name="psbd")
                        for c in range(nCT):
                            nc.tensor.matmul(
                                psb[:, c * CHT:(c + 1) * CHT],
                                qrr_t[base:base + hd, qi * P:(qi + 1) * P],
                                rkT[pr][base:base + hd, c * CHT:(c + 1) * CHT],
                                start=True, stop=True,
                            )
                        bdo = bdoutp.tile([P, T + 1], BF16, name="bdo")
                        nc.vector.memset(bdo[:, 0:1], 0.0)
                        nc.vector.tensor_copy(bdo[:, 1:T + 1], psb[:])
                        eng = nc.sync if h == 0 else nc.scalar
                        eng.dma_start(
                            bdd_map[pr][h][qi * P:(qi + 1) * P, :], bdo[:])

                def rd(pr, h, kb):
                    """Transposing read of one [k,q] block of the shifted bd
                    matrix (sync HWDGE only -- scalar corrupts transposes)."""
                    bdT_t = bdTp.tile([P, T], BF16, name="bdT_t")
                    nc.sync.dma_start_transpose(
                        bdT_t[:], views_map[pr][h][:, kb * P:(kb + 1) * P])
                    bdTs_map[pr][h][kb] = bdT_t

                # deferred Z-normalization:  ctxp <- cstage * (1/Z)
                pending_norm = []

                def flush_norms():
                    while pending_norm:
                        pr_, h_, cstage_, zb_ = pending_norm.pop(0)
                        base = h_ * hd
                        with nc.allow_low_precision(reason="ctx bf16"):
                            nc.vector.tensor_mul(
                                ctxp[pr_][base:base + hd, :],
                                cstage_[0:hd, :], zb_[:])

                def head_begin(pr, h):
                    return {"psc": psC.tile([hd + 1, T], F32, name="psc"),
                            "pv": None}

                def emit_pv(st, pr, h, kb, pT_t):
                    habs = pr * hpb + h
                    for c in range(nCT):
                        cs = slice(c * CHT, (c + 1) * CHT)
                        nc.tensor.matmul(
                            st["psc"][:, cs],
                            Vsb[kb][:, habs * (hd + 1):(habs + 1) * (hd + 1)],
                            pT_t[:, cs],
                            start=(kb == 0), stop=(kb == nT - 1),
                        )

                def attn_step(st, pr, h, kb):
                    base = h * hd
                    bdT_t = bdTs_map[pr][h][kb]
                    pss = psS.tile([P, T], F32, tag="s", name="pss")
                    for c in range(nCT):
                        cs = slice(c * CHT, (c + 1) * CHT)
                        nc.tensor.matmul(
                            pss[:, cs],
                            Kt[pr][base:base + hd, kb * P:(kb + 1) * P],
                            Qrw[pr][base:base + hd, cs],
                            start=True, stop=False,
                        )
                    for c in range(nCT):
                        cs = slice(c * CHT, (c + 1) * CHT)
                        nc.tensor.matmul(
                            pss[:, cs], id_sb[:], bdT_t[:, cs],
                            start=False, stop=True,
                        )
                    pT_t = pTp.tile([P, T], BF16, name="pT_t")
                    nc.scalar.activation(pT_t[:], pss[:], Act.Exp)
                    if st["pv"] is not None:
                        emit_pv(st, pr, h, kb - 1, st["pv"])
                    st["pv"] = pT_t

                def head_end(st, pr, h):
                    emit_pv(st, pr, h, nT - 1, st["pv"])
                    psc = st["psc"]
                    # Z-normalization, entirely off the tensor engine:
                    # aligned copy off PSUM (cross-partition 64->0), fast
                    # approximate reciprocal (exact to ~3e-6), GpSimd
                    # partition-broadcast; context multiply deferred a pair.
                    cstage = ctxsp.tile([hd + 1, T], BF16, name="cstage")
                    nc.vector.tensor_copy(cstage[:], psc[:])
                    z0 = zsmp.tile([1, T], F32, tag="z0", name="z0")
                    nc.vector.tensor_copy(z0[0:1, :], psc[hd:hd + 1, :])
                    zi = zsmp.tile([1, T], F32, tag="zi", name="zi")
                    nc.vector.reciprocal_approx_fast(zi[0:1, :], z0[0:1, :])
                    zb = zbp.tile([hd, T], F32, name="zb")
                    nc.gpsimd.partition_broadcast(zb[:], zi[0:1, :], channels=hd)
                    pending_norm.append((pr, h, cstage, zb))

                # Main pipeline, four sections per iteration pr:
                #   A: bd(pr) qi=0..3          (PE light, DVE casts flow)
                #   B: attn(pr-1, h0) kb=0..7  (PE heavy; sync weaves pr-1's
                #                               h1 reads, consumed in D)
                #   C: bd(pr) qi=4..7
                #   D: attn(pr-1, h1) kb=0..7  (sync weaves pr's h0 reads,
                #                               consumed next iteration's B)
                # The DVE cast chain of A/C hides under B/D's attention
                # compute, and the sync queue never lumps or head-of-line
                # blocks the reads behind not-yet-ready writes.
                for pr in range(npair + 1):
                    has_bd = pr < npair
                    prev = pr - 1
                    if has_bd:
                        bdd_map[pr] = [dramp.tile([T, T + 1], BF16,
                                                  name="bddram")
                                       for _ in range(hpb)]
                        views_map[pr] = [shifted_view(bdd_map[pr][h])
                                         for h in range(hpb)]
                        bdTs_map[pr] = [[None] * nT for _ in range(hpb)]
                        qrr_t = qrrp.tile([P, T], BF16, name="qrr_t")
                        with nc.allow_low_precision(reason="bf16 activations"):
                            nc.vector.tensor_scalar_add(
                                qrr_t[:], Qrw[pr][:], drr_t[pr][:])
                    flush_norms()
                    if has_bd:
                        for qi in range(nT // 2):
                            bd_step(pr, qrr_t, qi)
                    if prev >= 0:
                        st0 = head_begin(prev, 0)
                        for i in range(nT):
                            rd(prev, 1, i)
                            attn_step(st0, prev, 0, i)
                        head_end(st0, prev, 0)
                    if has_bd:
                        for qi in range(nT // 2, nT):
                            bd_step(pr, qrr_t, qi)
                    st1 = head_begin(prev, 1) if prev >= 0 else None
                    for i in range(nT):
                        if prev >= 0:
                            attn_step(st1, prev, 1, i)
                        if has_bd:
                            rd(pr, 0, i)
                    if prev >= 0:
                        head_end(st1, prev, 1)
                flush_norms()

            # ---------------- Phase 4: output projection -------------------
            with (
                tc.tile_pool(name="ps4", bufs=2, space=bass.MemorySpace.PSUM) as psp4,
                tc.tile_pool(name="outb", bufs=3) as outp,
            ):
                for jo in range(nH):
                    pss = psp4.tile([P, T], F32, name="ps4t")
                    for pr in range(npair):
                        for c in range(nCT):
                            nc.tensor.matmul(
                                pss[:, c * CHT:(c + 1) * CHT],
                                wo_sb[jo][:, pr * P:(pr + 1) * P],
                                ctxp[pr][:, c * CHT:(c + 1) * CHT],
                                start=(pr == 0),
                                stop=(pr == npair - 1),
                            )
                    ot = outp.tile([P, T], F32)
                    nc.vector.tensor_scalar_add(ot[:], pss[:], bo_t[jo][:])
                    nc.scalar.dma_start(outT_d[jo * P:(jo + 1) * P, :], ot[:])

    nc.compile()
    return nc


def prep_inputs(inputs, T, H, heads):
    """Host-side prep: returns list of per-core in_map dicts."""
    hd = H // heads
    nH = H // P
    npair = heads // 2
    scale = hd ** -0.5
    B = inputs["inputs"].shape[1]
    bf16 = ml_dtypes.bfloat16

    x = np.asarray(inputs["inputs"], np.float32)          # [T, B, H]
    pos = np.asarray(inputs["pos"], np.float32)[:, 0, :]  # [Lr, H]
    Win = np.asarray(inputs["input_weights"], np.float32)  # [3H, H]
    bin_ = np.asarray(inputs["input_biases"], np.float32)  # [3H]
    Wp = np.asarray(inputs["pos_weights"], np.float32)     # [H, H]
    bp = np.asarray(inputs["pos_biases"], np.float32)      # [H]
    Wo = np.asarray(inputs["output_weights"], np.float32)  # [H, H]
    bo = np.asarray(inputs["output_biases"], np.float32)   # [H]
    r_i = np.asarray(inputs["r_i"], np.float32)
    s_i = np.asarray(inputs["s_i"], np.float32)
    r_p = np.asarray(inputs["r_p"], np.float32)
    s_p = np.asarray(inputs["s_p"], np.float32)
    rw = np.asarray(inputs["r_w_bias"], np.float32)        # [heads, hd]
    rr = np.asarray(inputs["r_r_bias"], np.float32)        # [heads, hd]

    posT = np.ascontiguousarray(pos.T).astype(bf16)        # [H, Lr]
    ident = np.eye(P, dtype=bf16)

    b3 = bin_.reshape(heads, 3, hd)
    bq = ((b3[:, 0, :] + rw) * scale).reshape(H)
    bk = b3[:, 1, :].reshape(H)
    bv = b3[:, 2, :].reshape(H)
    drr = (scale * (rr - rw)).reshape(H)
    bo_eff = bo + Wo @ bv

    def pack_w(WT):
        # [H(in), H(out)] -> [nH(jo), P, H]:  [jo][p, kb*P+m] = WT[kb*P+p, jo*P+m]
        t = WT.reshape(nH, P, nH, P).transpose(2, 1, 0, 3)
        return np.ascontiguousarray(t.reshape(nH, P, H)).astype(bf16)

    def tile_bias(v):  # [H] -> [nH, P, 1]
        return np.ascontiguousarray(v.reshape(nH, P, 1))

    WoT = np.ascontiguousarray(Wo.T)  # [H, H]
    # [nH(jo), P(i=pair dims), H]:  [jo][i, pr*P+m] = WoT[pr*P+i, jo*P+m]
    wo_t = np.ascontiguousarray(
        WoT.reshape(npair, P, nH, P).transpose(2, 1, 0, 3).reshape(nH, P, H)
    ).astype(bf16)

    in_maps = []
    for b in range(B):
        WeffT = (Win.T * r_i[b][:, None]) * s_i[b][None, :]   # [H, 3H]
        We = WeffT.reshape(H, heads, 3, hd)
        WqT = np.ascontiguousarray(We[:, :, 0, :].reshape(H, H) * scale)
        WkT = np.ascontiguousarray(We[:, :, 1, :].reshape(H, H))
        WvT = np.ascontiguousarray(We[:, :, 2, :].reshape(H, H))
        WpT = (Wp.T * r_p[b][:, None]) * s_p[b][None, :]      # [H, H]
        in_maps.append({
            "xT": np.ascontiguousarray(x[:, b, :].T).astype(bf16),
            "posT": posT,
            "wq": pack_w(WqT),
            "wk": pack_w(WkT),
            "wp": pack_w(np.ascontiguousarray(WpT)),
            "wv": np.ascontiguousarray(WvT.reshape(nH, P, H)).astype(bf16),
            "wo": wo_t,
            "bqrw": tile_bias(bq),
            "bk": tile_bias(bk),
            "bp": tile_bias(bp),
            "drr": tile_bias(drr),
            "bo": tile_bias(bo_eff),
            "ident": ident,
        })
    return in_maps


_CACHE = {}
LAST_RESULT = None


def _get_program(T, H, heads, num_devices):
    key = (T, H, heads, num_devices)
    if key not in _CACHE:
        _CACHE[key] = build_program(T, H, heads, num_devices=num_devices)
    return _CACHE[key]


def kernel(**inputs):
    global LAST_RESULT
    T, B, H = inputs["inputs"].shape
    heads = int(inputs["heads"])
    nc = _get_program(T, H, heads, num_devices=B)
    in_maps = prep_inputs(inputs, T, H, heads)
    res = run_bass_kernel_spmd(nc, in_maps, core_ids=list(range(B)))
    LAST_RESULT = res
    out = np.stack([res.results[b]["outT"].T for b in range(B)], axis=1)
    return np.ascontiguousarray(out.astype(np.float32))


def run_profiled(**inputs):
    """Like kernel() but with trace=True; returns (out, BassKernelResults)."""
    global LAST_RESULT
    T, B, H = inputs["inputs"].shape
    heads = int(inputs["heads"])
    nc = _get_program(T, H, heads, num_devices=B)
    in_maps = prep_inputs(inputs, T, H, heads)
    res = run_bass_kernel_spmd(nc, in_maps, core_ids=list(range(B)), trace=True)
    LAST_RESULT = res
    out = np.stack([res.results[b]["outT"].T for b in range(B)], axis=1)
    return np.ascontiguousarray(out.astype(np.float32)), res


# revision 31
# speedup vs baseline: 1.1122x; 1.1122x over previous
"""Trainium2 Bass kernel for BERelativeSelfMultiheadAttn.

Strategy (data-parallel over batch B=8, one batch per NeuronCore):
  - Host folds the BatchEnsemble scale vectors r_*/s_* and the attention
    scale 1/sqrt(hd) into per-batch effective weight matrices (bf16),
    transposes x/pos, and packs weights so every DMA row is a 2KB
    contiguous chunk (fat descriptors).
  - On device, Q/K/rk are computed transposed ([feature, t]) so the
    score matmuls need no on-chip transposes; V is computed in [t, feature]
    layout with an extra ones column that yields the softmax normalizer Z
    for free from the PV matmul.
  - The relative shift is done exactly via the classic flat-buffer trick:
    bd [T, Lr] is written to DRAM as [T, 1+Lr] rows (zero in col 0); the
    shifted matrix is flat[T : T+T*T].reshape(T, T).  It is read back with a
    transposing DMA (bf16) and accumulated into the score PSUM with an
    identity matmul.
  - Softmax skips the max-subtraction (scores are O(1); exp cannot
    overflow).  The normalizer 1/Z is computed with a DVE reciprocal and
    broadcast across partitions by GpSimd (no activation-table swaps, no
    tensor-engine involvement); the context multiply is deferred by one
    head pair so no engine ever stalls on it.
  - Context is stored packed in head pairs [128, T] so the output
    projection runs K=128 matmuls.
  - DMA descriptor generation is spread across the two HWDGE queues:
    transposed reads on Sync, bd writes split Sync/Scalar, weight/x loads
    on Scalar.  Shifted-row reads for pair p are issued before pair p+1's
    bd writes to avoid head-of-line blocking.
"""

import numpy as np
import ml_dtypes

import concourse.bass as bass
import concourse.mybir as mybir
import concourse.tile as tile
from concourse import bacc
from concourse.bass_utils import run_bass_kernel_spmd

F32 = mybir.dt.float32
BF16 = mybir.dt.bfloat16
Act = mybir.ActivationFunctionType
Alu = mybir.AluOpType

P = 128


def build_program(T=1024, H=1024, heads=16, num_devices=8, enable_asserts=False):
    hd = H // heads
    assert hd == 64, "layout assumes head dim 64"
    nT = T // P            # t/q/k/r 128-blocks
    nH = H // P            # hidden-feature 128-blocks
    hpb = P // hd          # heads per 128-block (2)
    npair = heads // hpb
    CHT = min(512, T)      # matmul N chunk along T
    nCT = T // CHT
    CHH = min(512, H)      # matmul N chunk along H (V features)
    nCH_ = H // CHH
    Lr = T

    nc = bacc.Bacc(
        "TRN2",
        target_bir_lowering=False,
        debug=False,
        enable_asserts=enable_asserts,
        num_devices=num_devices,
    )

    xT_d = nc.dram_tensor("xT", [H, T], BF16, kind="ExternalInput").ap()
    posT_d = nc.dram_tensor("posT", [H, Lr], BF16, kind="ExternalInput").ap()
    wq_d = nc.dram_tensor("wq", [nH, P, H], BF16, kind="ExternalInput").ap()
    wk_d = nc.dram_tensor("wk", [nH, P, H], BF16, kind="ExternalInput").ap()
    wp_d = nc.dram_tensor("wp", [nH, P, H], BF16, kind="ExternalInput").ap()
    wv_d = nc.dram_tensor("wv", [nH, P, H], BF16, kind="ExternalInput").ap()
    wo_d = nc.dram_tensor("wo", [nH, P, H], BF16, kind="ExternalInput").ap()
    bqrw_d = nc.dram_tensor("bqrw", [nH, P, 1], F32, kind="ExternalInput").ap()
    bk_d = nc.dram_tensor("bk", [nH, P, 1], F32, kind="ExternalInput").ap()
    bp_d = nc.dram_tensor("bp", [nH, P, 1], F32, kind="ExternalInput").ap()
    drr_d = nc.dram_tensor("drr", [nH, P, 1], F32, kind="ExternalInput").ap()
    bo_d = nc.dram_tensor("bo", [nH, P, 1], F32, kind="ExternalInput").ap()
    ident_d = nc.dram_tensor("ident", [P, P], BF16, kind="ExternalInput").ap()
    outT_d = nc.dram_tensor("outT", [H, T], F32, kind="ExternalOutput").ap()

    with tile.TileContext(nc) as tc:
        with (
            tc.tile_pool(name="const", bufs=1) as constp,
            tc.tile_pool(name="persist", bufs=1) as pp,
        ):
            # constants
            id_sb = constp.tile([P, P], BF16, tag="ident")
            nc.sync.dma_start(id_sb[:], ident_d[:])
            bqrw_t, bk_t, bp_t, drr_t, bo_t = [], [], [], [], []
            for jo in range(nH):
                for lst, d, nm in (
                    (bqrw_t, bqrw_d, "bqrw"),
                    (bk_t, bk_d, "bk"),
                    (bp_t, bp_d, "bp"),
                    (drr_t, drr_d, "drr"),
                    (bo_t, bo_d, "bo"),
                ):
                    t = constp.tile([P, 1], F32, tag=f"{nm}{jo}", name=f"{nm}_{jo}")
                    nc.sync.dma_start(t[:], d[jo])
                    lst.append(t)

            # persistent activation tensors
            rkT = [pp.tile([P, Lr], BF16, tag=f"rk{i}", name=f"rkT{i}")
                   for i in range(nH)]
            Qrw = [pp.tile([P, T], BF16, tag=f"qrw{i}", name=f"Qrw{i}")
                   for i in range(nH)]
            Kt = [pp.tile([P, T], BF16, tag=f"kt{i}", name=f"Kt{i}")
                  for i in range(nH)]
            Vsb = [pp.tile([P, heads * (hd + 1)], BF16, tag=f"v{i}", name=f"Vsb{i}")
                   for i in range(nT)]
            # context packed by head PAIR: pair pr rows 0:64 = head 2pr,
            # rows 64:128 = head 2pr+1  ->  K=128 output projection
            ctxp = [pp.tile([P, T], BF16, tag=f"ctx{pr}", name=f"ctxp{pr}")
                    for pr in range(npair)]

            # -------- Phases 1+2: rk, Q, K (transposed), V (direct) --------
            with (
                tc.tile_pool(name="w12", bufs=1) as wpool,
                tc.tile_pool(name="xp", bufs=1) as xpool,
                tc.tile_pool(name="ps12", bufs=3, space=bass.MemorySpace.PSUM) as psp2,
            ):
                # Preload everything up front (scalar HWDGE queue) so the
                # tensor engine never starves and its clock stays ramped.
                posT_sb = [xpool.tile([P, Lr], BF16, tag=f"pos{i}",
                                      name=f"posT{i}") for i in range(nH)]
                xT_sb = [xpool.tile([P, T], BF16, tag=f"x{i}", name=f"xT{i}")
                         for i in range(nH)]
                wp_sb = [wpool.tile([P, H], BF16, tag=f"wp{i}", name=f"wp{i}")
                         for i in range(nH)]
                wq_sb = [wpool.tile([P, H], BF16, tag=f"wq{i}", name=f"wq{i}")
                         for i in range(nH)]
                wk_sb = [wpool.tile([P, H], BF16, tag=f"wk{i}", name=f"wk{i}")
                         for i in range(nH)]
                wv_sb = [wpool.tile([P, H], BF16, tag=f"wv{i}", name=f"wv{i}")
                         for i in range(nH)]
                for kb in range(nH):
                    nc.sync.dma_start(posT_sb[kb][:], posT_d[kb * P:(kb + 1) * P, :])
                    nc.scalar.dma_start(wp_sb[kb][:], wp_d[kb])
                for kb in range(nH):
                    nc.sync.dma_start(xT_sb[kb][:], xT_d[kb * P:(kb + 1) * P, :])
                    nc.scalar.dma_start(wq_sb[kb][:], wq_d[kb])
                for kb in range(nH):
                    nc.scalar.dma_start(wk_sb[kb][:], wk_d[kb])
                    nc.scalar.dma_start(wv_sb[kb][:], wv_d[kb])

                # rk projection
                for jo in range(nH):
                    pss = psp2.tile([P, T], F32, tag="ps12", name="ps1t")
                    for kb in range(nH):
                        for c in range(nCT):
                            nc.tensor.matmul(
                                pss[:, c * CHT:(c + 1) * CHT],
                                wp_sb[jo][:, kb * P:(kb + 1) * P],
                                posT_sb[kb][:, c * CHT:(c + 1) * CHT],
                                start=(kb == 0),
                                stop=(kb == nH - 1),
                            )
                    with nc.allow_low_precision(reason="bf16 activations"):
                        nc.vector.tensor_scalar_add(
                            rkT[jo][:], pss[:], bp_t[jo][:])

                # Q / K projections
                for jo in range(nH):
                    for w_sb, bias_t, dst in (
                        (wq_sb, bqrw_t, Qrw),
                        (wk_sb, bk_t, Kt),
                    ):
                        pss = psp2.tile([P, T], F32, tag="ps12", name="ps2t")
                        for kb in range(nH):
                            for c in range(nCT):
                                nc.tensor.matmul(
                                    pss[:, c * CHT:(c + 1) * CHT],
                                    w_sb[jo][:, kb * P:(kb + 1) * P],
                                    xT_sb[kb][:, c * CHT:(c + 1) * CHT],
                                    start=(kb == 0),
                                    stop=(kb == nH - 1),
                                )
                        with nc.allow_low_precision(reason="bf16 activations"):
                            nc.vector.tensor_scalar_add(
                                dst[jo][:], pss[:], bias_t[jo][:])

                # V (direct layout [t, feature] with interleaved ones col)
                for ti in range(nT):
                    nc.vector.memset(Vsb[ti][:], 1.0)
                    psv = psp2.tile([P, H], F32, tag="ps12", name="psvt")
                    for kb in range(nH):
                        for c in range(nCH_):
                            nc.tensor.matmul(
                                psv[:, c * CHH:(c + 1) * CHH],
                                xT_sb[kb][:, ti * P:(ti + 1) * P],
                                wv_sb[kb][:, c * CHH:(c + 1) * CHH],
                                start=(kb == 0),
                                stop=(kb == nH - 1),
                            )
                    # strided copy PSUM -> interleaved [V_h | 1] layout
                    nc.vector.tensor_copy(
                        Vsb[ti][:].rearrange(
                            "p (h e) -> p h e", e=hd + 1)[:, :, 0:hd],
                        psv[:].rearrange("p (h d) -> p h d", d=hd),
                    )

            # Preload the output-projection weights (packed per jo, pair
            # tiles side by side) so phase 4 never waits on DMA.
            wo_sb = [pp.tile([P, H], BF16, tag=f"wo{jo}", name=f"wo{jo}")
                     for jo in range(nH)]
            for jo in range(nH):
                nc.scalar.dma_start(wo_sb[jo][:], wo_d[jo])

            # ---------------- Phase 3: per head-pair attention -------------
            # Software-pipelined: shifted-row reads of pair p are issued
            # first, then bd scores of pair p+1, then the attention compute
            # of pair p, so the PE never drains at the bd -> DRAM ->
            # shifted-read dependency and the sync queue never head-of-line
            # blocks on not-yet-ready bd writes.
            with (
                tc.tile_pool(name="qrr", bufs=2) as qrrp,
                tc.tile_pool(name="bdout", bufs=6) as bdoutp,
                tc.tile_pool(name="bdT", bufs=16) as bdTp,
                tc.tile_pool(name="pT", bufs=4) as pTp,
                tc.tile_pool(name="ctxs", bufs=5) as ctxsp,
                tc.tile_pool(name="zsm", bufs=2) as zsmp,
                tc.tile_pool(name="zbp", bufs=4) as zbp,
                tc.tile_pool(name="psS", bufs=3, space=bass.MemorySpace.PSUM) as psS,
                tc.tile_pool(name="psC", bufs=1, space=bass.MemorySpace.PSUM) as psC,
                tc.tile_pool(name="bdd", bufs=6, space=bass.MemorySpace.DRAM) as dramp,
            ):
                def shifted_view(bdd_h):
                    flat = bdd_h[:].rearrange("a b -> (a b)")
                    return flat[T:T + T * T].rearrange("(a b) -> a b", b=T)

                bdd_map = {}    # pr -> [dram tile per head]
                views_map = {}  # pr -> [shifted view per head]
                bdTs_map = {}   # pr -> [[bdT tiles kb=0..7] per head]

                def emit_bd(pr, qrr_t):
                    """bd raw scores of pair pr + flat-buffer writes.  The
                    last few PSUM->bf16 casts go to the otherwise-idle GpSimd
                    so the DVE cast chain finishes sooner."""
                    for qi in range(nT):
                        for h in range(hpb):
                            base = h * hd
                            psb = psS.tile([P, T], F32, tag="s", name="psbd")
                            for c in range(nCT):
                                nc.tensor.matmul(
                                    psb[:, c * CHT:(c + 1) * CHT],
                                    qrr_t[base:base + hd, qi * P:(qi + 1) * P],
                                    rkT[pr][base:base + hd, c * CHT:(c + 1) * CHT],
                                    start=True, stop=True,
                                )
                            bdo = bdoutp.tile([P, T + 1], BF16, name="bdo")
                            nc.vector.memset(bdo[:, 0:1], 0.0)
                            nc.vector.tensor_copy(bdo[:, 1:T + 1], psb[:])
                            eng = nc.sync if h == 0 else nc.scalar
                            eng.dma_start(
                                bdd_map[pr][h][qi * P:(qi + 1) * P, :], bdo[:])

                def rd(pr, h, kb):
                    """Transposing read of one [k,q] block of the shifted bd
                    matrix (sync HWDGE only -- scalar corrupts transposes)."""
                    bdT_t = bdTp.tile([P, T], BF16, name="bdT_t")
                    nc.sync.dma_start_transpose(
                        bdT_t[:], views_map[pr][h][:, kb * P:(kb + 1) * P])
                    bdTs_map[pr][h][kb] = bdT_t

                # deferred Z-normalization:  ctxp <- cstage * (1/Z)
                pending_norm = []

                def flush_norms():
                    while pending_norm:
                        pr_, h_, cstage_, zb_ = pending_norm.pop(0)
                        base = h_ * hd
                        with nc.allow_low_precision(reason="ctx bf16"):
                            nc.vector.tensor_mul(
                                ctxp[pr_][base:base + hd, :],
                                cstage_[0:hd, :], zb_[:])

                def emit_attn(pr, read_pr):
                    """Attention of pair pr; transposing reads of pair
                    read_pr (consumed next pair) are woven one per kb step,
                    where the sync queue would otherwise idle."""
                    for h in range(hpb):
                        habs = pr * hpb + h
                        base = h * hd
                        psc = psC.tile([hd + 1, T], F32, name="psc")

                        def emit_pv(kb, pT_t):
                            for c in range(nCT):
                                cs = slice(c * CHT, (c + 1) * CHT)
                                nc.tensor.matmul(
                                    psc[:, cs],
                                    Vsb[kb][:, habs * (hd + 1):
                                            (habs + 1) * (hd + 1)],
                                    pT_t[:, cs],
                                    start=(kb == 0), stop=(kb == nT - 1),
                                )

                        pv_pending = None
                        for kb in range(nT):
                            if read_pr is not None:
                                rd(read_pr, h, kb)
                            bdT_t = bdTs_map[pr][h][kb]
                            pss = psS.tile([P, T], F32, tag="s", name="pss")
                            for c in range(nCT):
                                cs = slice(c * CHT, (c + 1) * CHT)
                                nc.tensor.matmul(
                                    pss[:, cs],
                                    Kt[pr][base:base + hd, kb * P:(kb + 1) * P],
                                    Qrw[pr][base:base + hd, cs],
                                    start=True, stop=False,
                                )
                            for c in range(nCT):
                                cs = slice(c * CHT, (c + 1) * CHT)
                                nc.tensor.matmul(
                                    pss[:, cs], id_sb[:], bdT_t[:, cs],
                                    start=False, stop=True,
                                )
                            pT_t = pTp.tile([P, T], BF16, name="pT_t")
                            nc.scalar.activation(pT_t[:], pss[:], Act.Exp)
                            if pv_pending is not None:
                                emit_pv(kb - 1, pv_pending)
                            pv_pending = pT_t
                        emit_pv(nT - 1, pv_pending)
                        # Z-normalization, entirely off the tensor engine:
                        # 1/Z on DVE, partition-broadcast on GpSimd, context
                        # multiply deferred until the next pair.
                        cstage = ctxsp.tile([hd + 1, T], BF16, name="cstage")
                        nc.vector.tensor_copy(cstage[:], psc[:])
                        # 1/Z: aligned copy off PSUM (cross-partition 64->0),
                        # then fast approximate reciprocal (exact to ~3e-6),
                        # then GpSimd partition-broadcast; the context multiply
                        # is deferred a pair so nothing stalls on it.
                        z0 = zsmp.tile([1, T], F32, tag="z0", name="z0")
                        nc.vector.tensor_copy(z0[0:1, :], psc[hd:hd + 1, :])
                        zi = zsmp.tile([1, T], F32, tag="zi", name="zi")
                        nc.vector.reciprocal_approx_fast(
                            zi[0:1, :], z0[0:1, :])
                        zb = zbp.tile([hd, T], F32, name="zb")
                        nc.gpsimd.partition_broadcast(
                            zb[:], zi[0:1, :], channels=hd)
                        pending_norm.append((pr, h, cstage, zb))

                for pr in range(npair):
                    bdd_map[pr] = [dramp.tile([T, T + 1], BF16, name="bddram")
                                   for _ in range(hpb)]
                    views_map[pr] = [shifted_view(bdd_map[pr][h])
                                     for h in range(hpb)]
                    bdTs_map[pr] = [[None] * nT for _ in range(hpb)]
                    qrr_t = qrrp.tile([P, T], BF16, name="qrr_t")
                    with nc.allow_low_precision(reason="bf16 activations"):
                        nc.vector.tensor_scalar_add(
                            qrr_t[:], Qrw[pr][:], drr_t[pr][:])
                    emit_bd(pr, qrr_t)
                    if pr == 0:
                        for h in range(hpb):
                            for kb in range(nT):
                                rd(0, h, kb)
                    flush_norms()
                    if pr > 0:
                        emit_attn(pr - 1, read_pr=pr)
                emit_attn(npair - 1, read_pr=None)
                flush_norms()

            # ---------------- Phase 4: output projection -------------------
            with (
                tc.tile_pool(name="ps4", bufs=2, space=bass.MemorySpace.PSUM) as psp4,
                tc.tile_pool(name="outb", bufs=3) as outp,
            ):
                for jo in range(nH):
                    pss = psp4.tile([P, T], F32, name="ps4t")
                    for pr in range(npair):
                        for c in range(nCT):
                            nc.tensor.matmul(
                                pss[:, c * CHT:(c + 1) * CHT],
                                wo_sb[jo][:, pr * P:(pr + 1) * P],
                                ctxp[pr][:, c * CHT:(c + 1) * CHT],
                                start=(pr == 0),
                                stop=(pr == npair - 1),
                            )
                    ot = outp.tile([P, T], F32)
                    nc.vector.tensor_scalar_add(ot[:], pss[:], bo_t[jo][:])
                    nc.scalar.dma_start(outT_d[jo * P:(jo + 1) * P, :], ot[:])

    nc.compile()
    return nc


def prep_inputs(inputs, T, H, heads):
    """Host-side prep: returns list of per-core in_map dicts."""
    hd = H // heads
    nH = H // P
    npair = heads // 2
    scale = hd ** -0.5
    B = inputs["inputs"].shape[1]
    bf16 = ml_dtypes.bfloat16

    x = np.asarray(inputs["inputs"], np.float32)          # [T, B, H]
    pos = np.asarray(inputs["pos"], np.float32)[:, 0, :]  # [Lr, H]
    Win = np.asarray(inputs["input_weights"], np.float32)  # [3H, H]
    bin_ = np.asarray(inputs["input_biases"], np.float32)  # [3H]
    Wp = np.asarray(inputs["pos_weights"], np.float32)     # [H, H]
    bp = np.asarray(inputs["pos_biases"], np.float32)      # [H]
    Wo = np.asarray(inputs["output_weights"], np.float32)  # [H, H]
    bo = np.asarray(inputs["output_biases"], np.float32)   # [H]
    r_i = np.asarray(inputs["r_i"], np.float32)
    s_i = np.asarray(inputs["s_i"], np.float32)
    r_p = np.asarray(inputs["r_p"], np.float32)
    s_p = np.asarray(inputs["s_p"], np.float32)
    rw = np.asarray(inputs["r_w_bias"], np.float32)        # [heads, hd]
    rr = np.asarray(inputs["r_r_bias"], np.float32)        # [heads, hd]

    posT = np.ascontiguousarray(pos.T).astype(bf16)        # [H, Lr]
    ident = np.eye(P, dtype=bf16)

    b3 = bin_.reshape(heads, 3, hd)
    bq = ((b3[:, 0, :] + rw) * scale).reshape(H)
    bk = b3[:, 1, :].reshape(H)
    bv = b3[:, 2, :].reshape(H)
    drr = (scale * (rr - rw)).reshape(H)
    bo_eff = bo + Wo @ bv

    def pack_w(WT):
        # [H(in), H(out)] -> [nH(jo), P, H]:  [jo][p, kb*P+m] = WT[kb*P+p, jo*P+m]
        t = WT.reshape(nH, P, nH, P).transpose(2, 1, 0, 3)
        return np.ascontiguousarray(t.reshape(nH, P, H)).astype(bf16)

    def tile_bias(v):  # [H] -> [nH, P, 1]
        return np.ascontiguousarray(v.reshape(nH, P, 1))

    WoT = np.ascontiguousarray(Wo.T)  # [H, H]
    # [nH(jo), P(i=pair dims), H]:  [jo][i, pr*P+m] = WoT[pr*P+i, jo*P+m]
    wo_t = np.ascontiguousarray(
        WoT.reshape(npair, P, nH, P).transpose(2, 1, 0, 3).reshape(nH, P, H)
    ).astype(bf16)

    in_maps = []
    for b in range(B):
        WeffT = (Win.T * r_i[b][:, None]) * s_i[b][None, :]   # [H, 3H]
        We = WeffT.reshape(H, heads, 3, hd)
        WqT = np.ascontiguousarray(We[:, :, 0, :].reshape(H, H) * scale)
        WkT = np.ascontiguousarray(We[:, :, 1, :].reshape(H, H))
        WvT = np.ascontiguousarray(We[:, :, 2, :].reshape(H, H))
        WpT = (Wp.T * r_p[b][:, None]) * s_p[b][None, :]      # [H, H]
        in_maps.append({
            "xT": np.ascontiguousarray(x[:, b, :].T).astype(bf16),
            "posT": posT,
            "wq": pack_w(WqT),
            "wk": pack_w(WkT),
            "wp": pack_w(np.ascontiguousarray(WpT)),
            "wv": np.ascontiguousarray(WvT.reshape(nH, P, H)).astype(bf16),
            "wo": wo_t,
            "bqrw": tile_bias(bq),
            "bk": tile_bias(bk),
            "bp": tile_bias(bp),
            "drr": tile_bias(drr),
            "bo": tile_bias(bo_eff),
            "ident": ident,
        })
    return in_maps


_CACHE = {}
LAST_RESULT = None


def _get_program(T, H, heads, num_devices):
    key = (T, H, heads, num_devices)
    if key not in _CACHE:
        _CACHE[key] = build_program(T, H, heads, num_devices=num_devices)
    return _CACHE[key]


def kernel(**inputs):
    global LAST_RESULT
    T, B, H = inputs["inputs"].shape
    heads = int(inputs["heads"])
    nc = _get_program(T, H, heads, num_devices=B)
    in_maps = prep_inputs(inputs, T, H, heads)
    res = run_bass_kernel_spmd(nc, in_maps, core_ids=list(range(B)))
    LAST_RESULT = res
    out = np.stack([res.results[b]["outT"].T for b in range(B)], axis=1)
    return np.ascontiguousarray(out.astype(np.float32))


def run_profiled(**inputs):
    """Like kernel() but with trace=True; returns (out, BassKernelResults)."""
    global LAST_RESULT
    T, B, H = inputs["inputs"].shape
    heads = int(inputs["heads"])
    nc = _get_program(T, H, heads, num_devices=B)
    in_maps = prep_inputs(inputs, T, H, heads)
    res = run_bass_kernel_spmd(nc, in_maps, core_ids=list(range(B)), trace=True)
    LAST_RESULT = res
    out = np.stack([res.results[b]["outT"].T for b in range(B)], axis=1)
    return np.ascontiguousarray(out.astype(np.float32)), res


# revision 35
# speedup vs baseline: 1.3962x; 1.2554x over previous
"""Trainium2 Bass kernel for BERelativeSelfMultiheadAttn.

Strategy (data-parallel over batch B=8, one batch per NeuronCore):
  - Host folds the BatchEnsemble scale vectors r_*/s_* and the attention
    scale 1/sqrt(hd) into per-batch effective weight matrices (bf16),
    transposes x/pos, and packs weights so every DMA row is a 2KB
    contiguous chunk (fat descriptors).
  - On device, Q/K/rk are computed transposed ([feature, t]) so the
    score matmuls need no on-chip transposes; V is computed in [t, feature]
    layout with an extra ones column that yields the softmax normalizer Z
    for free from the PV matmul.
  - The relative shift is done exactly via the classic flat-buffer trick:
    bd [T, Lr] is written to DRAM as [T, 1+Lr] rows (zero in col 0); the
    shifted matrix is flat[T : T+T*T].reshape(T, T).  It is read back with a
    transposing DMA (bf16, sync queue only) and accumulated into the score
    PSUM with an identity matmul.
  - Softmax skips the max-subtraction (scores are O(1); exp cannot
    overflow).  The normalizer 1/Z uses an aligned PSUM copy + fast
    approximate reciprocal on DVE and a GpSimd partition-broadcast; the
    context multiply is deferred by one head pair so nothing stalls on it.
  - Context is stored packed in head pairs [128, T] so the output
    projection runs K=128 matmuls.
  - Pair 0's bd scores and shifted-row reads are hoisted into the
    projection phase (woven between Q/K blocks and the V loop) so the
    attention pipeline is already warm when phase 3 begins.  Weights
    stream through a small rotating pool; projections, bd scores and
    attention share one PSUM pool so the phases can overlap.
"""

import numpy as np
import ml_dtypes

import concourse.bass as bass
import concourse.mybir as mybir
import concourse.tile as tile
from concourse import bacc
from concourse.bass_utils import run_bass_kernel_spmd

F32 = mybir.dt.float32
BF16 = mybir.dt.bfloat16
Act = mybir.ActivationFunctionType
Alu = mybir.AluOpType

P = 128


def build_program(T=1024, H=1024, heads=16, num_devices=8, enable_asserts=False):
    hd = H // heads
    assert hd == 64, "layout assumes head dim 64"
    nT = T // P            # t/q/k/r 128-blocks
    nH = H // P            # hidden-feature 128-blocks
    hpb = P // hd          # heads per 128-block (2)
    npair = heads // hpb
    CHT = min(512, T)      # matmul N chunk along T
    nCT = T // CHT
    CHH = min(512, H)      # matmul N chunk along H (V features)
    nCH_ = H // CHH
    Lr = T

    nc = bacc.Bacc(
        "TRN2",
        target_bir_lowering=False,
        debug=False,
        enable_asserts=enable_asserts,
        num_devices=num_devices,
    )

    xT_d = nc.dram_tensor("xT", [H, T], BF16, kind="ExternalInput").ap()
    posT_d = nc.dram_tensor("posT", [H, Lr], BF16, kind="ExternalInput").ap()
    wq_d = nc.dram_tensor("wq", [nH, P, H], BF16, kind="ExternalInput").ap()
    wk_d = nc.dram_tensor("wk", [nH, P, H], BF16, kind="ExternalInput").ap()
    wp_d = nc.dram_tensor("wp", [nH, P, H], BF16, kind="ExternalInput").ap()
    wv_d = nc.dram_tensor("wv", [nH, P, H], BF16, kind="ExternalInput").ap()
    wo_d = nc.dram_tensor("wo", [nH, P, H], BF16, kind="ExternalInput").ap()
    bqrw_d = nc.dram_tensor("bqrw", [nH, P, 1], F32, kind="ExternalInput").ap()
    bk_d = nc.dram_tensor("bk", [nH, P, 1], F32, kind="ExternalInput").ap()
    bp_d = nc.dram_tensor("bp", [nH, P, 1], F32, kind="ExternalInput").ap()
    drr_d = nc.dram_tensor("drr", [nH, P, 1], F32, kind="ExternalInput").ap()
    bo_d = nc.dram_tensor("bo", [nH, P, 1], F32, kind="ExternalInput").ap()
    ident_d = nc.dram_tensor("ident", [P, P], BF16, kind="ExternalInput").ap()
    outT_d = nc.dram_tensor("outT", [H, T], F32, kind="ExternalOutput").ap()

    with tile.TileContext(nc) as tc:
        with (
            tc.tile_pool(name="const", bufs=1) as constp,
            tc.tile_pool(name="persist", bufs=1) as pp,
            tc.tile_pool(name="qrr", bufs=2) as qrrp,
            tc.tile_pool(name="bdout", bufs=6) as bdoutp,
            tc.tile_pool(name="bdT", bufs=16) as bdTp,
            tc.tile_pool(name="psS", bufs=3, space=bass.MemorySpace.PSUM) as psS,
            tc.tile_pool(name="psC", bufs=1, space=bass.MemorySpace.PSUM) as psC,
            tc.tile_pool(name="bdd", bufs=6, space=bass.MemorySpace.DRAM) as dramp,
        ):
            # constants
            id_sb = constp.tile([P, P], BF16, tag="ident")
            nc.sync.dma_start(id_sb[:], ident_d[:])
            bqrw_t, bk_t, bp_t, drr_t, bo_t = [], [], [], [], []
            for jo in range(nH):
                for lst, d, nm in (
                    (bqrw_t, bqrw_d, "bqrw"),
                    (bk_t, bk_d, "bk"),
                    (bp_t, bp_d, "bp"),
                    (drr_t, drr_d, "drr"),
                    (bo_t, bo_d, "bo"),
                ):
                    t = constp.tile([P, 1], F32, tag=f"{nm}{jo}", name=f"{nm}_{jo}")
                    nc.sync.dma_start(t[:], d[jo])
                    lst.append(t)

            # persistent activation tensors
            rkT = [pp.tile([P, Lr], BF16, tag=f"rk{i}", name=f"rkT{i}")
                   for i in range(nH)]
            Qrw = [pp.tile([P, T], BF16, tag=f"qrw{i}", name=f"Qrw{i}")
                   for i in range(nH)]
            Kt = [pp.tile([P, T], BF16, tag=f"kt{i}", name=f"Kt{i}")
                  for i in range(nH)]
            Vsb = [pp.tile([P, heads * (hd + 1)], BF16, tag=f"v{i}", name=f"Vsb{i}")
                   for i in range(nT)]
            # context packed by head PAIR: pair pr rows 0:64 = head 2pr,
            # rows 64:128 = head 2pr+1  ->  K=128 output projection
            ctxp = [pp.tile([P, T], BF16, tag=f"ctx{pr}", name=f"ctxp{pr}")
                    for pr in range(npair)]

            # -------- shared bd machinery (used from phase 1/2 on) --------
            bdd_map = {}    # pr -> [dram tile per head]
            views_map = {}  # pr -> [shifted view per head]
            bdTs_map = {}   # pr -> [[bdT tiles kb=0..7] per head]

            def shifted_view(bdd_h):
                flat = bdd_h[:].rearrange("a b -> (a b)")
                return flat[T:T + T * T].rearrange("(a b) -> a b", b=T)

            def new_pair(pr):
                bdd_map[pr] = [dramp.tile([T, T + 1], BF16, name="bddram")
                               for _ in range(hpb)]
                views_map[pr] = [shifted_view(bdd_map[pr][h])
                                 for h in range(hpb)]
                bdTs_map[pr] = [[None] * nT for _ in range(hpb)]
                qrr_t = qrrp.tile([P, T], BF16, name="qrr_t")
                with nc.allow_low_precision(reason="bf16 activations"):
                    nc.vector.tensor_scalar_add(
                        qrr_t[:], Qrw[pr][:], drr_t[pr][:])
                return qrr_t

            def bd_qi(pr, qrr_t, qi):
                """bd raw scores of (pr, qi) for both heads + flat writes."""
                for h in range(hpb):
                    base = h * hd
                    psb = psS.tile([P, T], F32, tag="s", name="psbd")
                    for c in range(nCT):
                        nc.tensor.matmul(
                            psb[:, c * CHT:(c + 1) * CHT],
                            qrr_t[base:base + hd, qi * P:(qi + 1) * P],
                            rkT[pr][base:base + hd, c * CHT:(c + 1) * CHT],
                            start=True, stop=True,
                        )
                    bdo = bdoutp.tile([P, T + 1], BF16, name="bdo")
                    nc.vector.memset(bdo[:, 0:1], 0.0)
                    nc.vector.tensor_copy(bdo[:, 1:T + 1], psb[:])
                    eng = nc.sync if h == 0 else nc.scalar
                    eng.dma_start(
                        bdd_map[pr][h][qi * P:(qi + 1) * P, :], bdo[:])

            def rd(pr, h, kb):
                """Transposing read of one [k,q] block of the shifted bd
                matrix (sync HWDGE only -- scalar corrupts transposes)."""
                bdT_t = bdTp.tile([P, T], BF16, name="bdT_t")
                nc.sync.dma_start_transpose(
                    bdT_t[:], views_map[pr][h][:, kb * P:(kb + 1) * P])
                bdTs_map[pr][h][kb] = bdT_t

            # -------- Phases 1+2: rk, Q, K (transposed), V (direct) --------
            # bd(0) is woven between the Q/K blocks and its shifted-row
            # reads into the V loop, so phase 3 starts with a warm pipeline.
            with (
                tc.tile_pool(name="pos", bufs=1) as pospool,
                tc.tile_pool(name="wm", bufs=12) as wmats,
                tc.tile_pool(name="xp", bufs=1) as xpool,
            ):
                posT_sb = [pospool.tile([P, Lr], BF16, tag=f"pos{i}",
                                        name=f"posT{i}") for i in range(nH)]
                xT_sb = [xpool.tile([P, T], BF16, tag=f"x{i}", name=f"xT{i}")
                         for i in range(nH)]
                for kb in range(nH):
                    nc.sync.dma_start(
                        posT_sb[kb][:], posT_d[kb * P:(kb + 1) * P, :])
                for kb in range(nH):
                    nc.sync.dma_start(xT_sb[kb][:], xT_d[kb * P:(kb + 1) * P, :])

                # weights stream through a rotating pool (scalar queue):
                # wp0..7, then wq/wk interleaved, then wv0..7.
                wtile = {}

                def wload(mat, d, j):
                    t = wmats.tile([P, H], BF16, tag="w", name=f"w{mat}{j}")
                    nc.scalar.dma_start(t[:], d[j])
                    wtile[(mat, j)] = t

                load_seq = ([("p", wp_d, j) for j in range(nH)]
                            + [(m, d, j) for j in range(nH)
                               for (m, d) in (("q", wq_d), ("k", wk_d))]
                            + [("v", wv_d, j) for j in range(nH)])
                for _ in range(12):
                    wload(*load_seq.pop(0))

                def wnext(n=2):
                    for _ in range(n):
                        if load_seq:
                            wload(*load_seq.pop(0))

                def proj(w_t, src_sb, bias_t, dst):
                    pss = psS.tile([P, T], F32, tag="s", name="psproj")
                    for kb in range(nH):
                        for c in range(nCT):
                            nc.tensor.matmul(
                                pss[:, c * CHT:(c + 1) * CHT],
                                w_t[:, kb * P:(kb + 1) * P],
                                src_sb[kb][:, c * CHT:(c + 1) * CHT],
                                start=(kb == 0),
                                stop=(kb == nH - 1),
                            )
                    with nc.allow_low_precision(reason="bf16 activations"):
                        nc.vector.tensor_scalar_add(dst[:], pss[:], bias_t[:])

                # rk projection
                for jo in range(nH):
                    proj(wtile[("p", jo)][:], posT_sb, bp_t[jo][:], rkT[jo][:])
                    wnext(2)

                # Q/K jo=0, then bd(0) qi-blocks woven between later Q/K
                proj(wtile[("q", 0)][:], xT_sb, bqrw_t[0][:], Qrw[0][:])
                proj(wtile[("k", 0)][:], xT_sb, bk_t[0][:], Kt[0][:])
                qrr0 = new_pair(0)
                for jo in range(1, nH):
                    if jo <= nT // 2:
                        bd_qi(0, qrr0, 2 * (jo - 1))
                        bd_qi(0, qrr0, 2 * (jo - 1) + 1)
                    proj(wtile[("q", jo)][:], xT_sb, bqrw_t[jo][:], Qrw[jo][:])
                    wnext(2)
                    proj(wtile[("k", jo)][:], xT_sb, bk_t[jo][:], Kt[jo][:])
                    wnext(2)

                # V (direct layout with interleaved ones col); pair 0's
                # shifted-row reads woven 2 per t-block.
                for ti in range(nT):
                    nc.vector.memset(Vsb[ti][:], 1.0)
                    psv = psS.tile([P, H], F32, tag="s", name="psvt")
                    for kb in range(nH):
                        for c in range(nCH_):
                            nc.tensor.matmul(
                                psv[:, c * CHH:(c + 1) * CHH],
                                xT_sb[kb][:, ti * P:(ti + 1) * P],
                                wtile[("v", kb)][:, c * CHH:(c + 1) * CHH],
                                start=(kb == 0),
                                stop=(kb == nH - 1),
                            )
                    nc.vector.tensor_copy(
                        Vsb[ti][:].rearrange(
                            "p (h e) -> p h e", e=hd + 1)[:, :, 0:hd],
                        psv[:].rearrange("p (h d) -> p h d", d=hd),
                    )
                    rd(0, 0, ti)
                    rd(0, 1, ti)

            # Preload the output-projection weights (packed per jo, pair
            # tiles side by side) so phase 4 never waits on DMA.
            wo_sb = [pp.tile([P, H], BF16, tag=f"wo{jo}", name=f"wo{jo}")
                     for jo in range(nH)]
            for jo in range(nH):
                nc.scalar.dma_start(wo_sb[jo][:], wo_d[jo])

            # ---------------- Phase 3: per head-pair attention -------------
            with (
                tc.tile_pool(name="pT", bufs=4) as pTp,
                tc.tile_pool(name="ctxs", bufs=5) as ctxsp,
                tc.tile_pool(name="zsm", bufs=2) as zsmp,
                tc.tile_pool(name="zbp", bufs=4) as zbp,
            ):
                # deferred Z-normalization:  ctxp <- cstage * (1/Z)
                pending_norm = []

                def flush_norms():
                    while pending_norm:
                        pr_, h_, cstage_, zb_ = pending_norm.pop(0)
                        base = h_ * hd
                        with nc.allow_low_precision(reason="ctx bf16"):
                            nc.vector.tensor_mul(
                                ctxp[pr_][base:base + hd, :],
                                cstage_[0:hd, :], zb_[:])

                def emit_bd(pr, qrr_t, weave_prev):
                    """bd scores of pair pr; reads of pair pr-1 woven into
                    the qi loop so the sync queue never lumps."""
                    for qi in range(nT):
                        bd_qi(pr, qrr_t, qi)
                        if weave_prev:
                            rd(pr - 1, 0, qi)
                            rd(pr - 1, 1, qi)

                def emit_attn(pr):
                    for h in range(hpb):
                        habs = pr * hpb + h
                        base = h * hd
                        psc = psC.tile([hd + 1, T], F32, name="psc")

                        def emit_pv(kb, pT_t):
                            for c in range(nCT):
                                cs = slice(c * CHT, (c + 1) * CHT)
                                nc.tensor.matmul(
                                    psc[:, cs],
                                    Vsb[kb][:, habs * (hd + 1):
                                            (habs + 1) * (hd + 1)],
                                    pT_t[:, cs],
                                    start=(kb == 0), stop=(kb == nT - 1),
                                )

                        pv_pending = None
                        for kb in range(nT):
                            bdT_t = bdTs_map[pr][h][kb]
                            pss = psS.tile([P, T], F32, tag="s", name="pss")
                            for c in range(nCT):
                                cs = slice(c * CHT, (c + 1) * CHT)
                                nc.tensor.matmul(
                                    pss[:, cs],
                                    Kt[pr][base:base + hd, kb * P:(kb + 1) * P],
                                    Qrw[pr][base:base + hd, cs],
                                    start=True, stop=False,
                                )
                            for c in range(nCT):
                                cs = slice(c * CHT, (c + 1) * CHT)
                                nc.tensor.matmul(
                                    pss[:, cs], id_sb[:], bdT_t[:, cs],
                                    start=False, stop=True,
                                )
                            pT_t = pTp.tile([P, T], BF16, name="pT_t")
                            nc.scalar.activation(pT_t[:], pss[:], Act.Exp)
                            if pv_pending is not None:
                                emit_pv(kb - 1, pv_pending)
                            pv_pending = pT_t
                        emit_pv(nT - 1, pv_pending)
                        # Z-normalization, entirely off the tensor engine:
                        # aligned copy off PSUM (cross-partition 64->0), fast
                        # approximate reciprocal (exact to ~3e-6), GpSimd
                        # partition-broadcast; context multiply deferred.
                        cstage = ctxsp.tile([hd + 1, T], BF16, name="cstage")
                        nc.vector.tensor_copy(cstage[:], psc[:])
                        z0 = zsmp.tile([1, T], F32, tag="z0", name="z0")
                        nc.vector.tensor_copy(z0[0:1, :], psc[hd:hd + 1, :])
                        zi = zsmp.tile([1, T], F32, tag="zi", name="zi")
                        nc.vector.reciprocal_approx_fast(
                            zi[0:1, :], z0[0:1, :])
                        zb = zbp.tile([hd, T], F32, name="zb")
                        nc.gpsimd.partition_broadcast(
                            zb[:], zi[0:1, :], channels=hd)
                        pending_norm.append((pr, h, cstage, zb))

                for pr in range(1, npair):
                    qrr_t = new_pair(pr)
                    emit_bd(pr, qrr_t, weave_prev=(pr > 1))
                    flush_norms()
                    emit_attn(pr - 1)
                # tail: last pair's reads then its attention
                for h in range(hpb):
                    for kb in range(nT):
                        rd(npair - 1, h, kb)
                emit_attn(npair - 1)
                flush_norms()

            # ---------------- Phase 4: output projection -------------------
            with (
                tc.tile_pool(name="outb", bufs=3) as outp,
            ):
                for jo in range(nH):
                    pss = psS.tile([P, T], F32, tag="s", name="ps4t")
                    for pr in range(npair):
                        for c in range(nCT):
                            nc.tensor.matmul(
                                pss[:, c * CHT:(c + 1) * CHT],
                                wo_sb[jo][:, pr * P:(pr + 1) * P],
                                ctxp[pr][:, c * CHT:(c + 1) * CHT],
                                start=(pr == 0),
                                stop=(pr == npair - 1),
                            )
                    ot = outp.tile([P, T], F32)
                    with nc.allow_low_precision(reason="out bias"):
                        nc.vector.tensor_scalar_add(ot[:], pss[:], bo_t[jo][:])
                    nc.scalar.dma_start(outT_d[jo * P:(jo + 1) * P, :], ot[:])

    nc.compile()
    return nc


def prep_inputs(inputs, T, H, heads):
    """Host-side prep: returns list of per-core in_map dicts."""
    hd = H // heads
    nH = H // P
    npair = heads // 2
    scale = hd ** -0.5
    B = inputs["inputs"].shape[1]
    bf16 = ml_dtypes.bfloat16

    x = np.asarray(inputs["inputs"], np.float32)          # [T, B, H]
    pos = np.asarray(inputs["pos"], np.float32)[:, 0, :]  # [Lr, H]
    Win = np.asarray(inputs["input_weights"], np.float32)  # [3H, H]
    bin_ = np.asarray(inputs["input_biases"], np.float32)  # [3H]
    Wp = np.asarray(inputs["pos_weights"], np.float32)     # [H, H]
    bp = np.asarray(inputs["pos_biases"], np.float32)      # [H]
    Wo = np.asarray(inputs["output_weights"], np.float32)  # [H, H]
    bo = np.asarray(inputs["output_biases"], np.float32)   # [H]
    r_i = np.asarray(inputs["r_i"], np.float32)
    s_i = np.asarray(inputs["s_i"], np.float32)
    r_p = np.asarray(inputs["r_p"], np.float32)
    s_p = np.asarray(inputs["s_p"], np.float32)
    rw = np.asarray(inputs["r_w_bias"], np.float32)        # [heads, hd]
    rr = np.asarray(inputs["r_r_bias"], np.float32)        # [heads, hd]

    posT = np.ascontiguousarray(pos.T).astype(bf16)        # [H, Lr]
    ident = np.eye(P, dtype=bf16)

    b3 = bin_.reshape(heads, 3, hd)
    bq = ((b3[:, 0, :] + rw) * scale).reshape(H)
    bk = b3[:, 1, :].reshape(H)
    bv = b3[:, 2, :].reshape(H)
    drr = (scale * (rr - rw)).reshape(H)
    bo_eff = bo + Wo @ bv

    def pack_w(WT):
        # [H(in), H(out)] -> [nH(jo), P, H]:  [jo][p, kb*P+m] = WT[kb*P+p, jo*P+m]
        t = WT.reshape(nH, P, nH, P).transpose(2, 1, 0, 3)
        return np.ascontiguousarray(t.reshape(nH, P, H)).astype(bf16)

    def tile_bias(v):  # [H] -> [nH, P, 1]
        return np.ascontiguousarray(v.reshape(nH, P, 1))

    WoT = np.ascontiguousarray(Wo.T)  # [H, H]
    # [nH(jo), P(i=pair dims), H]:  [jo][i, pr*P+m] = WoT[pr*P+i, jo*P+m]
    wo_t = np.ascontiguousarray(
        WoT.reshape(npair, P, nH, P).transpose(2, 1, 0, 3).reshape(nH, P, H)
    ).astype(bf16)

    in_maps = []
    for b in range(B):
        WeffT = (Win.T * r_i[b][:, None]) * s_i[b][None, :]   # [H, 3H]
        We = WeffT.reshape(H, heads, 3, hd)
        WqT = np.ascontiguousarray(We[:, :, 0, :].reshape(H, H) * scale)
        WkT = np.ascontiguousarray(We[:, :, 1, :].reshape(H, H))
        WvT = np.ascontiguousarray(We[:, :, 2, :].reshape(H, H))
        WpT = (Wp.T * r_p[b][:, None]) * s_p[b][None, :]      # [H, H]
        in_maps.append({
            "xT": np.ascontiguousarray(x[:, b, :].T).astype(bf16),
            "posT": posT,
            "wq": pack_w(WqT),
            "wk": pack_w(WkT),
            "wp": pack_w(np.ascontiguousarray(WpT)),
            "wv": np.ascontiguousarray(WvT.reshape(nH, P, H)).astype(bf16),
            "wo": wo_t,
            "bqrw": tile_bias(bq),
            "bk": tile_bias(bk),
            "bp": tile_bias(bp),
            "drr": tile_bias(drr),
            "bo": tile_bias(bo_eff),
            "ident": ident,
        })
    return in_maps


_CACHE = {}
LAST_RESULT = None


def _get_program(T, H, heads, num_devices):
    key = (T, H, heads, num_devices)
    if key not in _CACHE:
        _CACHE[key] = build_program(T, H, heads, num_devices=num_devices)
    return _CACHE[key]


def kernel(**inputs):
    global LAST_RESULT
    T, B, H = inputs["inputs"].shape
    heads = int(inputs["heads"])
    nc = _get_program(T, H, heads, num_devices=B)
    in_maps = prep_inputs(inputs, T, H, heads)
    res = run_bass_kernel_spmd(nc, in_maps, core_ids=list(range(B)))
    LAST_RESULT = res
    out = np.stack([res.results[b]["outT"].T for b in range(B)], axis=1)
    return np.ascontiguousarray(out.astype(np.float32))


def run_profiled(**inputs):
    """Like kernel() but with trace=True; returns (out, BassKernelResults)."""
    global LAST_RESULT
    T, B, H = inputs["inputs"].shape
    heads = int(inputs["heads"])
    nc = _get_program(T, H, heads, num_devices=B)
    in_maps = prep_inputs(inputs, T, H, heads)
    res = run_bass_kernel_spmd(nc, in_maps, core_ids=list(range(B)), trace=True)
    LAST_RESULT = res
    out = np.stack([res.results[b]["outT"].T for b in range(B)], axis=1)
    return np.ascontiguousarray(out.astype(np.float32)), res
